# revision 9
# baseline (speedup 1.0000x reference)
"""Trainium2 Bass kernel for nn_DDA_PU_loss.

loss = sum((A-B)[pos]^2) * (1-alpha)/2 + sum((A-B)[neg]^2) * alpha/2
with A = drug_virus_reconstruct [8192, 16384], B = drug_virus [8192, 16384],
pos/neg given as 524288 / 2097152 random (x, y) int64 index pairs.
(drug_virus_mask is unused by the reference.)

Strategy (data-parallel row-shard):
  * Row-shard A, B into 8 blocks of 1024 rows (one per NeuronCore).
  * Host-side index prep (index-only, no value compute): bucket the index
    pairs by row-block and bincount them into per-cell multiplicities;
    build a sparse f32 weight-root matrix
        m = sqrt(wp * count_pos + wn * count_neg),
    wp = (1-alpha)/2, wn = alpha/2 — ~2% nonzero.
  * Device per core: stream A, B, m tiles through SBUF; DVE computes
    acc += ((A - B) * m)^2, which equals diff^2 * (wp*cpos + wn*cneg)
    per cell; final free-axis reduce -> [128, 1] partials.
  * Host sums the 8x128 partials -> scalar loss (already alpha-weighted).

This hits the memory roofline for the dense streams (3 x 64 MiB per core);
the gathered-sum formulation is exactly equivalent because the loss is a
multiplicity-weighted sum of squared diffs over cells.
"""

import numpy as np

N_DRUGS = 8192
N_VIRUS = 16384
N_CORES = 8
ROWS_PER_CORE = N_DRUGS // N_CORES  # 1024

FULL_CFG = dict(
    n_cores=N_CORES,
    rows_per_core=ROWS_PER_CORE,
    n_virus=N_VIRUS,
    tile_f=4096,   # free-dim tile size -> [128, 4096] f32 = 2 MiB per stream
)

TRACE = False
LAST_RESULTS = None

_BUILD_CACHE = {}


def build_nc(cfg):
    import concourse.tile as tile
    from concourse import bacc, mybir

    R = cfg["rows_per_core"]
    V = cfg["n_virus"]
    TF = cfg["tile_f"]
    n_rt = R // 128
    n_ft = V // TF

    nc = bacc.Bacc(
        "TRN2",
        target_bir_lowering=False,
        debug=False,
        num_devices=cfg["n_cores"],
    )
    a = nc.dram_tensor("a", [R, V], mybir.dt.float32, kind="ExternalInput").ap()
    b = nc.dram_tensor("b", [R, V], mybir.dt.float32, kind="ExternalInput").ap()
    m = nc.dram_tensor("m", [R, V], mybir.dt.float32, kind="ExternalInput").ap()
    partials = nc.dram_tensor(
        "partials", [128, 1], mybir.dt.float32, kind="ExternalOutput"
    ).ap()

    with tile.TileContext(nc) as tc:
        with tc.tile_pool(name="str", bufs=3) as spool, \
             tc.tile_pool(name="small", bufs=1) as small_pool:

            acc = small_pool.tile([128, TF], mybir.dt.float32)
            nc.vector.memset(acc[:], 0.0)

            for rt in range(n_rt):
                for ft in range(n_ft):
                    rsl = slice(rt * 128, rt * 128 + 128)
                    fsl = slice(ft * TF, (ft + 1) * TF)
                    at = spool.tile([128, TF], mybir.dt.float32, tag="at")
                    nc.sync.dma_start(out=at[:], in_=a[rsl, fsl])
                    bt = spool.tile([128, TF], mybir.dt.float32, tag="bt")
                    nc.scalar.dma_start(out=bt[:], in_=b[rsl, fsl])
                    mt = spool.tile([128, TF], mybir.dt.float32, tag="mt")
                    nc.gpsimd.dma_start(out=mt[:], in_=m[rsl, fsl])
                    # d = a - b (in-place into at)
                    nc.vector.tensor_tensor(
                        out=at[:], in0=at[:], in1=bt[:],
                        op=mybir.AluOpType.subtract,
                    )
                    # t = d * m (in-place into at)
                    nc.vector.tensor_tensor(
                        out=at[:], in0=at[:], in1=mt[:],
                        op=mybir.AluOpType.mult,
                    )
                    # sq = t * t (into bt)
                    nc.vector.tensor_tensor(
                        out=bt[:], in0=at[:], in1=at[:],
                        op=mybir.AluOpType.mult,
                    )
                    # acc += sq
                    nc.vector.tensor_tensor(
                        out=acc[:], in0=acc[:], in1=bt[:],
                        op=mybir.AluOpType.add,
                    )

            red = small_pool.tile([128, 1], mybir.dt.float32)
            nc.vector.tensor_reduce(
                out=red[:], in_=acc[:],
                axis=mybir.AxisListType.X, op=mybir.AluOpType.add,
            )
            nc.sync.dma_start(out=partials[:, :], in_=red[:])

    nc.compile()
    return nc


def build_masks(pos_x, pos_y, neg_x, neg_y, alpha, cfg):
    """Index-only host prep: per-core sqrt-weight matrices [R, V] f32."""
    R = cfg["rows_per_core"]
    V = cfg["n_virus"]
    n_cores = cfg["n_cores"]
    wp = (1.0 - float(alpha)) / 2.0
    wn = float(alpha) / 2.0
    px = np.asarray(pos_x).astype(np.int64, copy=False)
    py = np.asarray(pos_y).astype(np.int64, copy=False)
    nx = np.asarray(neg_x).astype(np.int64, copy=False)
    ny = np.asarray(neg_y).astype(np.int64, copy=False)
    pflat = px * V + py
    nflat = nx * V + ny
    pcore = px // R
    ncore = nx // R
    shard = R * V
    masks = []
    for c in range(n_cores):
        pl = pflat[pcore == c] - c * shard
        nl = nflat[ncore == c] - c * shard
        cp = np.bincount(pl, minlength=shard)
        cn = np.bincount(nl, minlength=shard)
        w = wp * cp.astype(np.float32) + wn * cn.astype(np.float32)
        np.sqrt(w, out=w)
        masks.append(w.reshape(R, V))
    return masks


def run_cores(in_maps, cfg):
    global LAST_RESULTS
    from concourse.bass_utils import run_bass_kernel_spmd
    from concourse.bass_interp import get_hw_module

    key = tuple(sorted(cfg.items()))
    if key not in _BUILD_CACHE:
        _BUILD_CACHE[key] = build_nc(cfg)
    nc = _BUILD_CACHE[key]

    old_m = nc.m
    nc.m = get_hw_module(nc.m)
    try:
        res = run_bass_kernel_spmd(
            nc,
            in_maps,
            core_ids=list(range(len(in_maps))),
            trace=TRACE,
        )
    finally:
        nc.m = old_m
    LAST_RESULTS = res
    return [r["partials"] for r in res.results]


def kernel(
    drug_virus_reconstruct,
    drug_virus,
    drug_virus_mask,
    pos_x_index,
    pos_y_index,
    neg_x_index,
    neg_y_index,
    alpha,
):
    cfg = FULL_CFG
    A = np.ascontiguousarray(np.asarray(drug_virus_reconstruct, dtype=np.float32))
    B = np.ascontiguousarray(np.asarray(drug_virus, dtype=np.float32))
    R = cfg["rows_per_core"]

    masks = build_masks(
        pos_x_index, pos_y_index, neg_x_index, neg_y_index, alpha, cfg
    )

    in_maps = [
        {
            "a": A[c * R: (c + 1) * R],
            "b": B[c * R: (c + 1) * R],
            "m": masks[c],
        }
        for c in range(cfg["n_cores"])
    ]

    partials = run_cores(in_maps, cfg)
    loss = float(
        np.sum([np.sum(p, dtype=np.float64) for p in partials], dtype=np.float64)
    )
    return np.float32(loss)


# revision 19
# speedup vs baseline: 18.4037x; 18.4037x over previous
"""Trainium2 Bass kernel for nn_DDA_PU_loss.

loss = sum((A-B)[pos]^2) * (1-alpha)/2 + sum((A-B)[neg]^2) * alpha/2
with A = drug_virus_reconstruct [8192, 16384], B = drug_virus [8192, 16384],
pos/neg given as 524288 / 2097152 random (x, y) int64 index pairs.
(drug_virus_mask is unused by the reference.)

Strategy (data-parallel row-shard):
  * Row-shard A, B into 8 blocks of 1024 rows (one per NeuronCore).
  * Host-side index prep (index-only, no value compute): bucket the index
    pairs by row-block and bincount them into per-cell multiplicities;
    build a sparse f32 weight-root matrix
        m = sqrt(wp * count_pos + wn * count_neg),
    wp = (1-alpha)/2, wn = alpha/2 — ~2% nonzero.
  * Device per core: stream A, B, m tiles through SBUF; DVE computes
    acc += ((A - B) * m)^2, which equals diff^2 * (wp*cpos + wn*cneg)
    per cell; final free-axis reduce -> [128, 1] partials.
  * Host sums the 8x128 partials -> scalar loss (already alpha-weighted).

This hits the memory roofline for the dense streams (3 x 64 MiB per core);
the gathered-sum formulation is exactly equivalent because the loss is a
multiplicity-weighted sum of squared diffs over cells.
"""

import numpy as np

N_DRUGS = 8192
N_VIRUS = 16384
N_CORES = 8
ROWS_PER_CORE = N_DRUGS // N_CORES  # 1024

FULL_CFG = dict(
    n_cores=N_CORES,
    rows_per_core=ROWS_PER_CORE,
    n_virus=N_VIRUS,
    tile_f=4096,   # free-dim tile size -> [128, 4096] f32 = 2 MiB per stream
    mask_f16=True,  # stream the sqrt-weight mask as fp16 (halves its traffic)
)

TRACE = False
LAST_RESULTS = None

_BUILD_CACHE = {}


def build_nc(cfg):
    import concourse.tile as tile
    from concourse import bacc, mybir

    R = cfg["rows_per_core"]
    V = cfg["n_virus"]
    TF = cfg["tile_f"]
    n_rt = R // 128
    n_ft = V // TF

    nc = bacc.Bacc(
        "TRN2",
        target_bir_lowering=False,
        debug=False,
        num_devices=cfg["n_cores"],
    )
    mdt = mybir.dt.float16 if cfg.get("mask_f16") else mybir.dt.float32
    a = nc.dram_tensor("a", [R, V], mybir.dt.float32, kind="ExternalInput").ap()
    b = nc.dram_tensor("b", [R, V], mybir.dt.float32, kind="ExternalInput").ap()
    m = nc.dram_tensor("m", [R, V], mdt, kind="ExternalInput").ap()
    partials = nc.dram_tensor(
        "partials", [128, 1], mybir.dt.float32, kind="ExternalOutput"
    ).ap()

    with tile.TileContext(nc) as tc:
        with tc.tile_pool(name="str", bufs=3) as spool, \
             tc.tile_pool(name="small", bufs=1) as small_pool:

            acc = small_pool.tile([128, TF], mybir.dt.float32)
            nc.vector.memset(acc[:], 0.0)

            for _rep in range(cfg.get("repeat", 1)):
              for rt in range(n_rt):
                for ft in range(n_ft):
                    rsl = slice(rt * 128, rt * 128 + 128)
                    fsl = slice(ft * TF, (ft + 1) * TF)
                    at = spool.tile([128, TF], mybir.dt.float32, tag="at")
                    nc.sync.dma_start(out=at[:], in_=a[rsl, fsl])
                    bt = spool.tile([128, TF], mybir.dt.float32, tag="bt")
                    nc.scalar.dma_start(out=bt[:], in_=b[rsl, fsl])
                    mt = spool.tile([128, TF], mdt, tag="mt")
                    nc.gpsimd.dma_start(out=mt[:], in_=m[rsl, fsl])
                    # d = a - b (in-place into at)
                    nc.vector.tensor_tensor(
                        out=at[:], in0=at[:], in1=bt[:],
                        op=mybir.AluOpType.subtract,
                    )
                    # t = d * m (in-place into at)
                    nc.vector.tensor_tensor(
                        out=at[:], in0=at[:], in1=mt[:],
                        op=mybir.AluOpType.mult,
                    )
                    # sq = t * t (into bt)
                    nc.vector.tensor_tensor(
                        out=bt[:], in0=at[:], in1=at[:],
                        op=mybir.AluOpType.mult,
                    )
                    # acc += sq
                    nc.vector.tensor_tensor(
                        out=acc[:], in0=acc[:], in1=bt[:],
                        op=mybir.AluOpType.add,
                    )

            red = small_pool.tile([128, 1], mybir.dt.float32)
            nc.vector.tensor_reduce(
                out=red[:], in_=acc[:],
                axis=mybir.AxisListType.X, op=mybir.AluOpType.add,
            )
            nc.sync.dma_start(out=partials[:, :], in_=red[:])

    nc.compile()
    return nc


def build_masks(pos_x, pos_y, neg_x, neg_y, alpha, cfg):
    """Index-only host prep: per-core sqrt-weight matrices [R, V].

    Returns (masks, scale): the device computes sum(d^2 * m^2); the final
    loss is scale * sum(partials).  Weights are rescaled by the dominant
    class weight so that the vast majority of nonzero mask cells are
    exactly 1.0 — exactly representable in fp16 — making the fp16 mask
    essentially lossless for the dominant class.
    """
    R = cfg["rows_per_core"]
    V = cfg["n_virus"]
    n_cores = cfg["n_cores"]
    wp = (1.0 - float(alpha)) / 2.0
    wn = float(alpha) / 2.0
    px = np.asarray(pos_x).astype(np.int64, copy=False)
    py = np.asarray(pos_y).astype(np.int64, copy=False)
    nx = np.asarray(neg_x).astype(np.int64, copy=False)
    ny = np.asarray(neg_y).astype(np.int64, copy=False)
    # dominant weight-mass class defines the scale (mask value 1.0)
    mass_p = wp * len(px)
    mass_n = wn * len(nx)
    scale = wn if mass_n >= mass_p else wp
    if scale == 0.0:
        scale = max(wp, wn, 1e-30)
    pflat = px * V + py
    nflat = nx * V + ny
    pcore = px // R
    ncore = nx // R
    shard = R * V
    masks = []
    for c in range(n_cores):
        pl = pflat[pcore == c] - c * shard
        nl = nflat[ncore == c] - c * shard
        cp = np.bincount(pl, minlength=shard)
        cn = np.bincount(nl, minlength=shard)
        w = (wp / scale) * cp.astype(np.float32) + (wn / scale) * cn.astype(
            np.float32
        )
        if cfg.get("mask_f16"):
            # dithered fp16 sqrt-weights: pick between the two adjacent fp16
            # values of sqrt(w) per cell so that E[m_f16^2] == w exactly;
            # the per-cell rounding becomes zero-mean noise that averages
            # out over the ~300k nonzero cells (~1e-6 relative).
            nz = np.flatnonzero(w)
            wv = w[nz]
            m0 = np.sqrt(wv).astype(np.float16)
            w0 = m0.astype(np.float32) ** 2
            toward = np.where(w0 < wv, np.float16(np.inf), np.float16(0.0))
            malt = np.nextafter(m0, toward)
            walt = malt.astype(np.float32) ** 2
            denom = w0 - walt
            q = np.where(denom != 0, (wv - walt) / np.where(denom == 0, 1, denom), 1.0)
            nzu = nz.astype(np.uint64)
            u = (
                ((nzu * np.uint64(2654435761)) & np.uint64(0xFFFFFFFF)) >> np.uint64(16)
            ).astype(np.float64) / 65536.0
            mv = np.where(u < q, m0, malt)
            mf = np.zeros(shard, dtype=np.float16)
            mf[nz] = mv
            masks.append(mf.reshape(R, V))
        else:
            np.sqrt(w, out=w)
            masks.append(w.reshape(R, V))
    return masks, scale


def run_cores(in_maps, cfg):
    global LAST_RESULTS
    from concourse.bass_utils import run_bass_kernel_spmd
    from concourse.bass_interp import get_hw_module

    key = tuple(sorted(cfg.items()))
    if key not in _BUILD_CACHE:
        _BUILD_CACHE[key] = build_nc(cfg)
    nc = _BUILD_CACHE[key]

    old_m = nc.m
    nc.m = get_hw_module(nc.m)
    try:
        res = run_bass_kernel_spmd(
            nc,
            in_maps,
            core_ids=list(range(len(in_maps))),
            trace=TRACE,
        )
    finally:
        nc.m = old_m
    LAST_RESULTS = res
    return [r["partials"] for r in res.results]


def kernel(
    drug_virus_reconstruct,
    drug_virus,
    drug_virus_mask,
    pos_x_index,
    pos_y_index,
    neg_x_index,
    neg_y_index,
    alpha,
):
    cfg = FULL_CFG
    A = np.ascontiguousarray(np.asarray(drug_virus_reconstruct, dtype=np.float32))
    B = np.ascontiguousarray(np.asarray(drug_virus, dtype=np.float32))
    R = cfg["rows_per_core"]

    masks, scale = build_masks(
        pos_x_index, pos_y_index, neg_x_index, neg_y_index, alpha, cfg
    )

    in_maps = [
        {
            "a": A[c * R: (c + 1) * R],
            "b": B[c * R: (c + 1) * R],
            "m": masks[c],
        }
        for c in range(cfg["n_cores"])
    ]

    partials = run_cores(in_maps, cfg)
    loss = scale * float(
        np.sum([np.sum(p, dtype=np.float64) for p in partials], dtype=np.float64)
    )
    return np.float32(loss)
